# revision 80
# baseline (speedup 1.0000x reference)
"""Masked multi-head attention (B=8, N=1024, C=768, H=12) on 8 trn2 NeuronCores.

Sharding: pure data-parallel over batch - core i computes batch element i
end-to-end (qkv linear, masked softmax attention, output projection).
No collectives.

Device-side layout is fully "transposed attention":
  qkT   [2C, N]  (q/k heads as [D, N] blocks, produced directly by matmul)
  S.T   [m, n] per head (keys on partitions)  ->  softmax over partition dim
        handled with: no max-subtraction (scores are O(1)), row-sums via a
        ones-column appended to V in the P@V matmul.
  EV.T  [D+1, N] per head accumulated in PSUM; row D holds the softmax denom.

Scheduling: all bias adds are fused into Pool/DVE evacuations (no PE bias
matmuls except two tiny bias/indicator rows), softmax normalization
broadcasts each pair's reciprocal rows with one K=2 indicator matmul and
scales attnT in place on DVE, qkv weight tiles are computed just-in-time inside the attention
phase to fill the PE gaps left by the ACT-paced exp stream, the first
projection row starts during the last two heads, and input DMA is split
across three rings in consumption order (first tiles packed contiguously
for minimum descriptor count).
"""

import numpy as np
import ml_dtypes

import concourse.bass as bass
import concourse.mybir as mybir
import concourse.tile as tile
from concourse import bacc
from concourse.bass_utils import run_bass_kernel_spmd

B, N, C, H = 8, 1024, 768, 12
D = C // H  # 64
SCALE = 0.125
NT = N // 128  # 8 n-tiles
CT = C // 128  # 6 c-tiles
BF16 = mybir.dt.bfloat16
F32 = mybir.dt.float32
NPBF16 = ml_dtypes.bfloat16

_CACHE: dict = {}


def _build_bass():
    nc = bacc.Bacc(None, target_bir_lowering=False, debug=False)

    xT_d = nc.dram_tensor("xT", [C, N], BF16, kind="ExternalInput")
    maskT_d = nc.dram_tensor("maskT", [N, N], BF16, kind="ExternalInput")
    wT_d = nc.dram_tensor("qkv_wT", [C, 3 * C], BF16, kind="ExternalInput")
    wqk0_d = nc.dram_tensor("wqk0", [128, 2 * C], BF16, kind="ExternalInput")
    pwT_d = nc.dram_tensor("proj_wT", [C, C], BF16, kind="ExternalInput")
    qb_d = nc.dram_tensor("qb_col", [128, 12], F32, kind="ExternalInput")
    vbb_d = nc.dram_tensor("vb_bcast", [128, C], BF16, kind="ExternalInput")
    pbb_d = nc.dram_tensor("pb_bcast", [128, C], BF16, kind="ExternalInput")
    out_d = nc.dram_tensor("out", [N, C], BF16, kind="ExternalOutput")

    with tile.TileContext(nc) as tc:
        _emit(nc, tc, xT_d, maskT_d, wT_d, wqk0_d, pwT_d, qb_d, vbb_d,
              pbb_d, out_d)
    nc.compile()
    return nc


def _emit(nc, tc, xT_d, maskT_d, wT_d, wqk0_d, pwT_d, qb_d, vbb_d, pbb_d,
          out_d):
    Exp = mybir.ActivationFunctionType.Exp
    Ident = mybir.ActivationFunctionType.Identity
    MUL = mybir.AluOpType.mult
    ADD = mybir.AluOpType.add

    with (
        tc.tile_pool(name="consts", bufs=1) as consts,
        tc.tile_pool(name="work", bufs=4) as work,
        tc.tile_pool(name="owork", bufs=2) as owork,
    ):
        # ---- persistent SBUF residents -------------------------------
        xT = consts.tile([128, CT, N], BF16, name="xT_sb")
        wT = consts.tile([128, CT, 3 * C], BF16, name="wT_sb")
        wqk0 = consts.tile([128, 2, C], BF16, name="wqk0_sb")
        pwT = consts.tile([128, CT, C], BF16, name="pwT_sb")
        maskT = consts.tile([128, NT, N], BF16, name="maskT_sb")
        qb = consts.tile([128, 12], F32, name="qb_sb")
        vbb = consts.tile([128, C], BF16, name="vbb_sb")
        pbb = consts.tile([128, C], BF16, name="pbb_sb")
        qkT = consts.tile([128, 12, N], BF16, name="qkT_sb")
        v_ext = consts.tile([128, NT, H * (D + 1)], BF16, name="vext_sb")
        attnT = consts.tile([128, CT, N], BF16, name="attnT_sb")
        rc2 = consts.tile([33, CT, N], BF16, name="rc2_sb")
        ind2 = consts.tile([33, 128], BF16, name="ind2_sb")
        ones = consts.tile([1, 128], BF16, name="ones_sb")

        v_ext_h = v_ext.rearrange("p t (h e) -> p t h e", e=D + 1)
        vbb_h = vbb.rearrange("p (h d) -> p h d", d=D)
        xT_r = xT_d.ap().rearrange("(t p) n -> p t n", p=128)
        wT_r = wT_d.ap().rearrange("(t p) n -> p t n", p=128)
        maskT_r = maskT_d.ap().rearrange("(t p) n -> p t n", p=128)
        pwT_r = pwT_d.ap().rearrange("(t p) n -> p t n", p=128)

        # ---- input DMA, consumption-ordered --------------------------
        for half in range(2):
            hs = slice(half * 512, (half + 1) * 512)
            for ct in range(3):
                nc.scalar.dma_start(out=xT[:, ct, hs], in_=xT_r[:, ct, hs])
            if half == 0:
                nc.sync.dma_start(out=wqk0, in_=wqk0_d.ap().rearrange(
                    "p (s n) -> p s n", s=2))
            for ct in range(3, CT):
                nc.sync.dma_start(out=xT[:, ct, hs], in_=xT_r[:, ct, hs])
        for ct in range(CT):
            nc.sync.dma_start(out=wT[:, ct, 2 * C:3 * C],
                              in_=wT_r[:, ct, 2 * C:3 * C])
        for p in range(1, 6):
            for c0 in (p * 128, 768 + p * 128):
                nc.sync.dma_start(out=wT[:, :, c0:c0 + 128],
                                  in_=wT_r[:, :, c0:c0 + 128])
        # Pool ring (cheap triggers): first mask rows + small consts
        nc.gpsimd.dma_start(out=maskT[:, 0, :], in_=maskT_r[:, 0, :])
        nc.gpsimd.dma_start(out=qb, in_=qb_d.ap())
        nc.gpsimd.dma_start(out=vbb, in_=vbb_d.ap())
        for j in range(1, 4):
            nc.gpsimd.dma_start(out=maskT[:, j, :], in_=maskT_r[:, j, :])
        # SP ring tail: later mask rows, proj consts
        for j in range(4, NT):
            nc.sync.dma_start(out=maskT[:, j, :], in_=maskT_r[:, j, :])
        nc.sync.dma_start(out=pbb, in_=pbb_d.ap())
        for ct in range(CT):
            nc.sync.dma_start(out=pwT[:, ct, :], in_=pwT_r[:, ct, :])

        # ones columns of v_ext (col D of each head block)
        nc.vector.memset(v_ext_h[:, :, :, D:D + 1], 1.0)
        nc.vector.memset(ones, 1.0)
        nc.vector.memset(ind2, 0.0)
        nc.vector.memset(rc2, 0.0)
        nc.vector.memset(ind2[0:1, 0:64], 1.0)
        nc.vector.memset(ind2[32:33, 64:128], 1.0)

        with (
            tc.tile_pool(name="psX", bufs=2, space="PSUM") as psX,
        ):
            def emit_qk_half(t, half, evac):
                # qkT tile t (q for t<6, k for t>=6), columns half*512..
                ps = psX.tile([128, 512], F32, name="psq", tag="psX")
                sl = slice(half * 512, (half + 1) * 512)
                for ct in range(CT):
                    if t == 0 or t == CT:
                        lhsT = wqk0[:, t // CT, ct * 128:(ct + 1) * 128]
                    else:
                        lhsT = wT[:, ct, t * 128:(t + 1) * 128]
                    nc.tensor.matmul(ps, lhsT, xT[:, ct, sl],
                                     start=(ct == 0), stop=(ct == CT - 1))
                if evac == "act":
                    nc.scalar.activation(out=qkT[:, t, sl], in_=ps,
                                         func=Ident, bias=qb[:, t:t + 1],
                                         scale=1.0)
                elif evac == "dve":
                    nc.vector.tensor_scalar_add(out=qkT[:, t, sl], in0=ps,
                                                scalar1=qb[:, t:t + 1])
                else:
                    nc.gpsimd.tensor_scalar_add(out=qkT[:, t, sl], in0=ps,
                                                scalar1=qb[:, t:t + 1])

            def emit_v_half(j, half):
                width = 512 if half == 0 else 256
                c0 = 2 * C + half * 512
                ps = psX.tile([128, width], F32, name="psv", tag="psX")
                for ct in range(CT):
                    nc.tensor.matmul(ps, xT[:, ct, j * 128:(j + 1) * 128],
                                     wT[:, ct, c0:c0 + width],
                                     start=(ct == 0), stop=(ct == CT - 1))
                h0 = half * 8
                hn = width // D
                nc.vector.tensor_copy(
                    out=v_ext_h[:, j, h0:h0 + hn, 0:D],
                    in_=ps.rearrange("p (h d) -> p h d", d=D))
                nc.gpsimd.tensor_add(
                    out=v_ext_h[:, j, h0:h0 + hn, 0:D],
                    in0=v_ext_h[:, j, h0:h0 + hn, 0:D],
                    in1=vbb_h[:, h0:h0 + hn, :])

            with (
                tc.tile_pool(name="psS", bufs=2, space="PSUM") as psSp,
                tc.tile_pool(name="psE", bufs=1, space="PSUM") as psEp,
            ):
                # prologue: q0 and k0 tiles through psX
                emit_qk_half(0, 0, "act")
                emit_qk_half(CT, 0, "dve")
                emit_qk_half(0, 1, "act")
                emit_qk_half(CT, 1, "dve")
                # (x halves arrive h0-first; prologue consumes in kind)

                em_tiles = {}

                def emit_S(h, j):
                    po = (h % 2) * 64
                    qt = h // 2
                    psS = psSp.tile([128, N], F32, name="psS", tag="psS")
                    kT_ap = qkT[po:po + D, CT + qt, j * 128:(j + 1) * 128]
                    for half in range(2):
                        nc.tensor.matmul(
                            psS[:, half * 512:(half + 1) * 512], kT_ap,
                            qkT[po:po + D, qt, half * 512:(half + 1) * 512],
                            start=True, stop=True)
                    e_sb = work.tile([128, N], BF16, name="e_sb", tag="e_sb")
                    nc.scalar.activation(out=e_sb, in_=psS, func=Exp,
                                         scale=SCALE)
                    em = work.tile([128, N], BF16, name="em_sb", tag="em_sb")
                    nc.vector.tensor_mul(out=em, in0=e_sb,
                                         in1=maskT[:, j, :])
                    em_tiles[(h, j)] = em

                def emit_EV(h, j, psE):
                    em = em_tiles.pop((h, j))
                    v_ap = v_ext[:, j, h * (D + 1):(h + 1) * (D + 1)]
                    for half in range(2):
                        nc.tensor.matmul(
                            psE[:, half * 512:(half + 1) * 512], v_ap,
                            em[:, half * 512:(half + 1) * 512],
                            start=(j == 0), stop=(j == NT - 1))

                def finish_head(h, psE):
                    po = (h % 2) * 64
                    qt = h // 2
                    pr = (h % 2) * 32
                    with nc.allow_low_precision(reason="softmax denom recip"):
                        if h == H - 1:
                            nc.vector.reciprocal(out=rc2[pr:pr + 1, qt, 0:128],
                                                 in_=psE[D:D + 1, 0:128])
                            nc.vector.reciprocal(
                                out=rc2[pr:pr + 1, qt, 128:1024],
                                in_=psE[D:D + 1, 128:1024])
                        else:
                            nc.vector.reciprocal(out=rc2[pr:pr + 1, qt, :],
                                                 in_=psE[D:D + 1, :])
                    if h < 9 and h % 2 == 0:
                        nc.vector.tensor_copy(out=attnT[po:po + D, qt, :],
                                              in_=psE[0:D, :])
                    else:
                        nc.scalar.copy(out=attnT[po:po + D, qt, :],
                                       in_=psE[0:D, :])

                def norm_cols(p, c0, c1, pool=None, tag="psX"):
                    # broadcast both heads' 1/denom rows over the pair's
                    # 128 partitions via a K=2 indicator matmul, then scale
                    # attnT in place from PSUM
                    psr = (pool or psX).tile([128, 512], F32, name="psr",
                                             tag=tag)
                    nc.tensor.matmul(psr[:, 0:c1 - c0], ind2,
                                     rc2[:, p, c0:c1],
                                     start=True, stop=True)
                    nc.vector.tensor_mul(out=attnT[:, p, c0:c1],
                                         in0=attnT[:, p, c0:c1],
                                         in1=psr[:, 0:c1 - c0])

                def norm_pair(p):
                    norm_cols(p, 0, 512)
                    norm_cols(p, 512, 1024)

                deferred = {(1, 0): [(1, 0)], (1, 2): [(1, 1)],
                            (1, 4): [(CT + 1, 0)], (1, 5): [(CT + 1, 1)]}
                for p in range(2, 6):
                    deferred[(2 * p - 2, 1)] = [(p, 0)]
                    deferred[(2 * p - 2, 4)] = [(p, 1)]
                    deferred[(2 * p - 1, 1)] = [(CT + p, 0)]
                    deferred[(2 * p - 1, 4)] = [(CT + p, 1)]

                proj_early = {
                    (10, 1): [(0, [0, 1])], (10, 4): [(0, [2, 3])],
                    (11, 1): [(0, [4]), (1, [0, 1])],
                    (11, 4): [(1, [2, 3, 4])],
                }

                steps = [(h, j) for h in range(H) for j in range(NT)]
                emit_S(0, 0)
                emit_S(0, 1)
                psE = None
                po_nt0 = {}

                for idx, (h, j) in enumerate(steps):
                    if j == 0:
                        psE = psEp.tile([D + 1, N], F32, name="psE",
                                        tag="psE")
                    for (t, half) in deferred.get((h, j), []):
                        if t < CT:
                            emit_qk_half(t, half, "dve" if half == 0 else "act")
                        else:
                            emit_qk_half(t, half, "dve" if half == 0 else "act")
                    if h == 0:
                        emit_v_half(j, 0)
                        emit_v_half(j, 1)
                    if j == 5 and h >= 2 and h % 2 == 0:
                        norm_pair(h // 2 - 1)
                    for (k, cts) in proj_early.get((h, j), []):
                        w0, wd = (0, 512) if k == 0 else (512, 256)
                        if k not in po_nt0:
                            po_nt0[k] = psX.tile([128, wd], F32, name="pso",
                                                 tag="psX")
                        for ct in cts:
                            nc.tensor.matmul(
                                po_nt0[k], attnT[:, ct, 0:128],
                                pwT[:, ct, w0:w0 + wd],
                                start=(ct == 0), stop=False)
                    emit_EV(h, j, psE)
                    if idx + 2 < len(steps):
                        emit_S(*steps[idx + 2])
                    if j == NT - 1:
                        finish_head(h, psE)

                # tail of attention scope: last head's norm, staged by
                # column blocks so nt0's ct5 unblocks almost immediately
                norm_cols(5, 0, 128, pool=psEp, tag="psE")
                norm_cols(5, 128, 512, pool=psEp, tag="psE")
                norm_cols(5, 512, 1024, pool=psEp, tag="psE")
                # nt0 ct5 + evac
                for k in range(2):
                    w0, wd = (0, 512) if k == 0 else (512, 256)
                    nc.tensor.matmul(po_nt0[k], attnT[:, CT - 1, 0:128],
                                     pwT[:, CT - 1, w0:w0 + wd],
                                     start=False, stop=False)
                    nc.tensor.matmul(po_nt0[k], ones,
                                     pbb[0:1, w0:w0 + wd],
                                     start=False, stop=True)
                oo0 = owork.tile([128, 512], BF16, name="oo0", tag="oo0")
                nc.scalar.copy(out=oo0, in_=po_nt0[0])
                nc.sync.dma_start(out=out_d.ap()[0:128, 0:512], in_=oo0)
                oo1 = owork.tile([128, 256], BF16, name="oo1", tag="oo1")
                nc.vector.tensor_copy(out=oo1, in_=po_nt0[1])
                nc.sync.dma_start(out=out_d.ap()[0:128, 512:768], in_=oo1)

            # ---- projection nt 1..7 (psS/psE banks now free) ----------
            with tc.tile_pool(name="psO", bufs=2, space="PSUM") as psOp:
                for nt in range(1, NT):
                    pso = psOp.tile([128, C], F32, name="pso", tag="pso")
                    sl = slice(nt * 128, (nt + 1) * 128)
                    for ct in range(CT):
                        lhsT = attnT[:, ct, sl]
                        nc.tensor.matmul(pso[:, 0:512], lhsT,
                                         pwT[:, ct, 0:512],
                                         start=(ct == 0), stop=False)
                        nc.tensor.matmul(pso[:, 512:768], lhsT,
                                         pwT[:, ct, 512:768],
                                         start=(ct == 0), stop=False)
                    nc.tensor.matmul(pso[:, 0:512], ones, pbb[0:1, 0:512],
                                     start=False, stop=True)
                    nc.tensor.matmul(pso[:, 512:768], ones,
                                     pbb[0:1, 512:768],
                                     start=False, stop=True)
                    oo0 = owork.tile([128, 512], BF16, name="oo0", tag="oo0")
                    oo1 = owork.tile([128, 256], BF16, name="oo1", tag="oo1")
                    if nt == NT - 1:
                        nc.vector.tensor_copy(out=oo0, in_=pso[:, 0:512])
                        nc.scalar.copy(out=oo1, in_=pso[:, 512:768])
                    else:
                        nc.scalar.copy(out=oo0, in_=pso[:, 0:512])
                        nc.vector.tensor_copy(out=oo1, in_=pso[:, 512:768])
                    nc.sync.dma_start(out=out_d.ap()[sl, 0:512], in_=oo0)
                    nc.sync.dma_start(out=out_d.ap()[sl, 512:768], in_=oo1)


def _host_prep_shared(qkv_w, qkv_b, proj_w, proj_b):
    wT = np.ascontiguousarray(qkv_w.T).astype(NPBF16)          # [C, 3C]
    pwT = np.ascontiguousarray(proj_w.T).astype(NPBF16)        # [C, C]
    blocks = []
    for ts in (0, 6):
        A = qkv_w[ts * 128:(ts + 1) * 128, :]          # [128c, C]
        blocks.append(np.ascontiguousarray(
            A.T.reshape(CT, 128, 128).transpose(1, 0, 2).reshape(128, C)))
    wqk0 = np.concatenate(blocks, axis=1).astype(NPBF16)   # [128, 2C]
    qb_col = np.ascontiguousarray(
        qkv_b[:2 * C].reshape(12, 128).T).astype(np.float32)
    vb_bcast = np.broadcast_to(qkv_b[2 * C:].astype(NPBF16),
                               (128, C)).copy()
    pb_bcast = np.broadcast_to(proj_b.astype(NPBF16), (128, C)).copy()
    return wT, wqk0, pwT, qb_col, vb_bcast, pb_bcast


def kernel(x, mask, qkv_w, qkv_b, proj_w, proj_b, _trace=False):
    if "nc" not in _CACHE:
        _CACHE["nc"] = _build_bass()
    nc = _CACHE["nc"]

    wT, wqk0, pwT, qb_col, vb_bcast, pb_bcast = _host_prep_shared(
        np.asarray(qkv_w), np.asarray(qkv_b), np.asarray(proj_w),
        np.asarray(proj_b))
    x = np.asarray(x)
    mask = np.asarray(mask)

    in_maps = []
    for i in range(B):
        in_maps.append({
            "xT": np.ascontiguousarray(x[i].T).astype(NPBF16),
            "maskT": np.ascontiguousarray(mask[i].T).astype(NPBF16),
            "qkv_wT": wT,
            "wqk0": wqk0,
            "proj_wT": pwT,
            "qb_col": qb_col,
            "vb_bcast": vb_bcast,
            "pb_bcast": pb_bcast,
        })
    res = run_bass_kernel_spmd(nc, in_maps, core_ids=list(range(B)),
                               trace=_trace)
    out = np.stack([np.asarray(res.results[i]["out"], dtype=np.float32)
                    for i in range(B)], axis=0)
    if _trace:
        _CACHE["last_results"] = res
    return out


# revision 85
# speedup vs baseline: 1.0033x; 1.0033x over previous
"""Masked multi-head attention (B=8, N=1024, C=768, H=12) on 8 trn2 NeuronCores.

Sharding: pure data-parallel over batch - core i computes batch element i
end-to-end (qkv linear, masked softmax attention, output projection).
No collectives.

Device-side layout is fully "transposed attention":
  qkT   [2C, N]  (q/k heads as [D, N] blocks, produced directly by matmul)
  S.T   [m, n] per head (keys on partitions)  ->  softmax over partition dim
        handled with: no max-subtraction (scores are O(1)), row-sums via a
        ones-column appended to V in the P@V matmul.
  EV.T  [D+1, N] per head accumulated in PSUM; row D holds the softmax denom.

Scheduling: all bias adds are fused into Pool/DVE evacuations (no PE bias
matmuls except two tiny bias/indicator rows), softmax normalization
broadcasts each pair's reciprocal rows with one K=2 indicator matmul and
scales attnT in place on DVE, qkv weight tiles are computed just-in-time inside the attention
phase to fill the PE gaps left by the ACT-paced exp stream, the first
projection row starts during the last two heads, and input DMA is split
across three rings in consumption order (first tiles packed contiguously
for minimum descriptor count).
"""

import numpy as np
import ml_dtypes

import concourse.bass as bass
import concourse.mybir as mybir
import concourse.tile as tile
from concourse import bacc
from concourse.bass_utils import run_bass_kernel_spmd

B, N, C, H = 8, 1024, 768, 12
D = C // H  # 64
SCALE = 0.125
NT = N // 128  # 8 n-tiles
CT = C // 128  # 6 c-tiles
BF16 = mybir.dt.bfloat16
F32 = mybir.dt.float32
NPBF16 = ml_dtypes.bfloat16

_CACHE: dict = {}


def _build_bass():
    nc = bacc.Bacc(None, target_bir_lowering=False, debug=False)

    xT_d = nc.dram_tensor("xT", [C, N], BF16, kind="ExternalInput")
    maskT_d = nc.dram_tensor("maskT", [N, N], BF16, kind="ExternalInput")
    wT_d = nc.dram_tensor("qkv_wT", [C, 3 * C], BF16, kind="ExternalInput")
    wqk0_d = nc.dram_tensor("wqk0", [128, 2 * C], BF16, kind="ExternalInput")
    pwT_d = nc.dram_tensor("proj_wT", [C, C], BF16, kind="ExternalInput")
    qb_d = nc.dram_tensor("qb_col", [128, 12], F32, kind="ExternalInput")
    vbb_d = nc.dram_tensor("vb_bcast", [128, C], BF16, kind="ExternalInput")
    pbb_d = nc.dram_tensor("pb_bcast", [128, C], BF16, kind="ExternalInput")
    out_d = nc.dram_tensor("out", [N, C], BF16, kind="ExternalOutput")

    with tile.TileContext(nc) as tc:
        _emit(nc, tc, xT_d, maskT_d, wT_d, wqk0_d, pwT_d, qb_d, vbb_d,
              pbb_d, out_d)
    nc.compile()
    return nc


def _emit(nc, tc, xT_d, maskT_d, wT_d, wqk0_d, pwT_d, qb_d, vbb_d, pbb_d,
          out_d):
    Exp = mybir.ActivationFunctionType.Exp
    Ident = mybir.ActivationFunctionType.Identity
    MUL = mybir.AluOpType.mult
    ADD = mybir.AluOpType.add

    with (
        tc.tile_pool(name="consts", bufs=1) as consts,
        tc.tile_pool(name="work", bufs=4) as work,
        tc.tile_pool(name="owork", bufs=3) as owork,
    ):
        # ---- persistent SBUF residents -------------------------------
        xT = consts.tile([128, CT, N], BF16, name="xT_sb")
        wT = consts.tile([128, CT, 3 * C], BF16, name="wT_sb")
        wqk0 = consts.tile([128, 2, C], BF16, name="wqk0_sb")
        pwT = consts.tile([128, CT, C], BF16, name="pwT_sb")
        maskT = consts.tile([128, NT, N], BF16, name="maskT_sb")
        qb = consts.tile([128, 12], F32, name="qb_sb")
        vbb = consts.tile([128, C], BF16, name="vbb_sb")
        pbb = consts.tile([128, C], BF16, name="pbb_sb")
        qkT = consts.tile([128, 12, N], BF16, name="qkT_sb")
        v_ext = consts.tile([128, NT, H * (D + 1)], BF16, name="vext_sb")
        attnT = consts.tile([128, CT, N], BF16, name="attnT_sb")
        rc2 = consts.tile([33, CT, N], BF16, name="rc2_sb")
        ind2 = consts.tile([33, 128], BF16, name="ind2_sb")
        ones = consts.tile([1, 128], BF16, name="ones_sb")

        v_ext_h = v_ext.rearrange("p t (h e) -> p t h e", e=D + 1)
        vbb_h = vbb.rearrange("p (h d) -> p h d", d=D)
        xT_r = xT_d.ap().rearrange("(t p) n -> p t n", p=128)
        wT_r = wT_d.ap().rearrange("(t p) n -> p t n", p=128)
        maskT_r = maskT_d.ap().rearrange("(t p) n -> p t n", p=128)
        pwT_r = pwT_d.ap().rearrange("(t p) n -> p t n", p=128)

        # ---- input DMA, consumption-ordered --------------------------
        for half in range(2):
            hs = slice(half * 512, (half + 1) * 512)
            for ct in range(3):
                nc.scalar.dma_start(out=xT[:, ct, hs], in_=xT_r[:, ct, hs])
            if half == 0:
                nc.sync.dma_start(out=wqk0, in_=wqk0_d.ap().rearrange(
                    "p (s n) -> p s n", s=2))
            for ct in range(3, CT):
                nc.sync.dma_start(out=xT[:, ct, hs], in_=xT_r[:, ct, hs])
        for ct in range(CT):
            nc.sync.dma_start(out=wT[:, ct, 2 * C:3 * C],
                              in_=wT_r[:, ct, 2 * C:3 * C])
        for p in range(1, 6):
            for c0 in (p * 128, 768 + p * 128):
                nc.sync.dma_start(out=wT[:, :, c0:c0 + 128],
                                  in_=wT_r[:, :, c0:c0 + 128])
        # Pool ring (cheap triggers): first mask rows + small consts
        nc.gpsimd.dma_start(out=maskT[:, 0, :], in_=maskT_r[:, 0, :])
        nc.gpsimd.dma_start(out=qb, in_=qb_d.ap())
        nc.gpsimd.dma_start(out=vbb, in_=vbb_d.ap())
        for j in range(1, 4):
            nc.gpsimd.dma_start(out=maskT[:, j, :], in_=maskT_r[:, j, :])
        # SP ring tail: later mask rows, proj consts
        for j in range(4, NT):
            nc.sync.dma_start(out=maskT[:, j, :], in_=maskT_r[:, j, :])
        nc.sync.dma_start(out=pbb, in_=pbb_d.ap())
        for ct in range(CT):
            nc.sync.dma_start(out=pwT[:, ct, :], in_=pwT_r[:, ct, :])

        # ones columns of v_ext (col D of each head block)
        nc.vector.memset(v_ext_h[:, :, :, D:D + 1], 1.0)
        nc.vector.memset(ones, 1.0)
        nc.vector.memset(ind2, 0.0)
        nc.vector.memset(rc2, 0.0)
        nc.vector.memset(ind2[0:1, 0:64], 1.0)
        nc.vector.memset(ind2[32:33, 64:128], 1.0)

        with (
            tc.tile_pool(name="psX", bufs=2, space="PSUM") as psX,
        ):
            def emit_qk_half(t, half, evac):
                # qkT tile t (q for t<6, k for t>=6), columns half*512..
                ps = psX.tile([128, 512], F32, name="psq", tag="psX")
                sl = slice(half * 512, (half + 1) * 512)
                for ct in range(CT):
                    if t == 0 or t == CT:
                        lhsT = wqk0[:, t // CT, ct * 128:(ct + 1) * 128]
                    else:
                        lhsT = wT[:, ct, t * 128:(t + 1) * 128]
                    nc.tensor.matmul(ps, lhsT, xT[:, ct, sl],
                                     start=(ct == 0), stop=(ct == CT - 1))
                if evac == "act":
                    nc.scalar.activation(out=qkT[:, t, sl], in_=ps,
                                         func=Ident, bias=qb[:, t:t + 1],
                                         scale=1.0)
                elif evac == "dve":
                    nc.vector.tensor_scalar_add(out=qkT[:, t, sl], in0=ps,
                                                scalar1=qb[:, t:t + 1])
                else:
                    nc.gpsimd.tensor_scalar_add(out=qkT[:, t, sl], in0=ps,
                                                scalar1=qb[:, t:t + 1])

            def emit_v_half(j, half):
                width = 512 if half == 0 else 256
                c0 = 2 * C + half * 512
                ps = psX.tile([128, width], F32, name="psv", tag="psX")
                for ct in range(CT):
                    nc.tensor.matmul(ps, xT[:, ct, j * 128:(j + 1) * 128],
                                     wT[:, ct, c0:c0 + width],
                                     start=(ct == 0), stop=(ct == CT - 1))
                h0 = half * 8
                hn = width // D
                nc.vector.tensor_copy(
                    out=v_ext_h[:, j, h0:h0 + hn, 0:D],
                    in_=ps.rearrange("p (h d) -> p h d", d=D))
                nc.gpsimd.tensor_add(
                    out=v_ext_h[:, j, h0:h0 + hn, 0:D],
                    in0=v_ext_h[:, j, h0:h0 + hn, 0:D],
                    in1=vbb_h[:, h0:h0 + hn, :])

            with (
                tc.tile_pool(name="psS", bufs=2, space="PSUM") as psSp,
                tc.tile_pool(name="psE", bufs=1, space="PSUM") as psEp,
            ):
                # prologue: q0 and k0 tiles through psX
                emit_qk_half(0, 0, "act")
                emit_qk_half(CT, 0, "dve")
                emit_qk_half(0, 1, "act")
                emit_qk_half(CT, 1, "dve")
                # (x halves arrive h0-first; prologue consumes in kind)

                em_tiles = {}

                def emit_S(h, j):
                    po = (h % 2) * 64
                    qt = h // 2
                    psS = psSp.tile([128, N], F32, name="psS", tag="psS")
                    kT_ap = qkT[po:po + D, CT + qt, j * 128:(j + 1) * 128]
                    for half in range(2):
                        nc.tensor.matmul(
                            psS[:, half * 512:(half + 1) * 512], kT_ap,
                            qkT[po:po + D, qt, half * 512:(half + 1) * 512],
                            start=True, stop=True)
                    e_sb = work.tile([128, N], BF16, name="e_sb", tag="e_sb")
                    nc.scalar.activation(out=e_sb, in_=psS, func=Exp,
                                         scale=SCALE)
                    em = work.tile([128, N], BF16, name="em_sb", tag="em_sb")
                    nc.vector.tensor_mul(out=em, in0=e_sb,
                                         in1=maskT[:, j, :])
                    em_tiles[(h, j)] = em

                def emit_EV(h, j, psE):
                    em = em_tiles.pop((h, j))
                    v_ap = v_ext[:, j, h * (D + 1):(h + 1) * (D + 1)]
                    for half in range(2):
                        nc.tensor.matmul(
                            psE[:, half * 512:(half + 1) * 512], v_ap,
                            em[:, half * 512:(half + 1) * 512],
                            start=(j == 0), stop=(j == NT - 1))

                def finish_head(h, psE):
                    po = (h % 2) * 64
                    qt = h // 2
                    pr = (h % 2) * 32
                    with nc.allow_low_precision(reason="softmax denom recip"):
                        if h == H - 1:
                            nc.vector.reciprocal(out=rc2[pr:pr + 1, qt, 0:128],
                                                 in_=psE[D:D + 1, 0:128])
                            nc.vector.reciprocal(
                                out=rc2[pr:pr + 1, qt, 128:1024],
                                in_=psE[D:D + 1, 128:1024])
                        else:
                            nc.vector.reciprocal(out=rc2[pr:pr + 1, qt, :],
                                                 in_=psE[D:D + 1, :])
                    if h < 9 and h % 2 == 0:
                        nc.vector.tensor_copy(out=attnT[po:po + D, qt, :],
                                              in_=psE[0:D, :])
                    else:
                        nc.scalar.copy(out=attnT[po:po + D, qt, :],
                                       in_=psE[0:D, :])

                def norm_cols(p, c0, c1, pool=None, tag="psX"):
                    # broadcast both heads' 1/denom rows over the pair's
                    # 128 partitions via a K=2 indicator matmul, then scale
                    # attnT in place from PSUM
                    psr = (pool or psX).tile([128, 512], F32, name="psr",
                                             tag=tag)
                    nc.tensor.matmul(psr[:, 0:c1 - c0], ind2,
                                     rc2[:, p, c0:c1],
                                     start=True, stop=True)
                    nc.vector.tensor_mul(out=attnT[:, p, c0:c1],
                                         in0=attnT[:, p, c0:c1],
                                         in1=psr[:, 0:c1 - c0])

                def norm_pair(p):
                    norm_cols(p, 0, 512)
                    norm_cols(p, 512, 1024)

                deferred = {(1, 0): [(1, 0)], (1, 2): [(1, 1)],
                            (1, 4): [(CT + 1, 0)], (1, 5): [(CT + 1, 1)]}
                for p in range(2, 6):
                    deferred[(2 * p - 2, 1)] = [(p, 0)]
                    deferred[(2 * p - 2, 4)] = [(p, 1)]
                    deferred[(2 * p - 1, 1)] = [(CT + p, 0)]
                    deferred[(2 * p - 1, 4)] = [(CT + p, 1)]

                proj_early = {
                    (10, 1): [(0, [0, 1])], (10, 4): [(0, [2, 3])],
                    (11, 1): [(0, [4]), (1, [0, 1])],
                    (11, 4): [(1, [2, 3, 4])],
                }

                steps = [(h, j) for h in range(H) for j in range(NT)]
                emit_S(0, 0)
                emit_S(0, 1)
                psE = None
                po_nt0 = {}

                for idx, (h, j) in enumerate(steps):
                    if j == 0:
                        psE = psEp.tile([D + 1, N], F32, name="psE",
                                        tag="psE")
                    for (t, half) in deferred.get((h, j), []):
                        if t < CT:
                            emit_qk_half(t, half, "dve" if half == 0 else "act")
                        else:
                            emit_qk_half(t, half, "dve" if half == 0 else "act")
                    if h == 0:
                        emit_v_half(j, 0)
                        emit_v_half(j, 1)
                    if j == 5 and h >= 2 and h % 2 == 0:
                        norm_pair(h // 2 - 1)
                    for (k, cts) in proj_early.get((h, j), []):
                        w0, wd = (0, 512) if k == 0 else (512, 256)
                        if k not in po_nt0:
                            po_nt0[k] = psX.tile([128, wd], F32, name="pso",
                                                 tag="psX")
                        for ct in cts:
                            nc.tensor.matmul(
                                po_nt0[k], attnT[:, ct, 0:128],
                                pwT[:, ct, w0:w0 + wd],
                                start=(ct == 0), stop=False)
                    emit_EV(h, j, psE)
                    if idx + 2 < len(steps):
                        emit_S(*steps[idx + 2])
                    if j == NT - 1:
                        finish_head(h, psE)

                # tail of attention scope: last head's norm, staged by
                # column blocks so nt0's ct5 unblocks almost immediately
                norm_cols(5, 0, 128, pool=psEp, tag="psE")
                norm_cols(5, 128, 512, pool=psEp, tag="psE")
                norm_cols(5, 512, 1024, pool=psEp, tag="psE")
                # nt0 ct5 + evac
                for k in range(2):
                    w0, wd = (0, 512) if k == 0 else (512, 256)
                    nc.tensor.matmul(po_nt0[k], attnT[:, CT - 1, 0:128],
                                     pwT[:, CT - 1, w0:w0 + wd],
                                     start=False, stop=False)
                    nc.tensor.matmul(po_nt0[k], ones,
                                     pbb[0:1, w0:w0 + wd],
                                     start=False, stop=True)
                oo0 = owork.tile([128, 512], BF16, name="oo0", tag="oo0")
                nc.scalar.copy(out=oo0, in_=po_nt0[0])
                nc.sync.dma_start(out=out_d.ap()[0:128, 0:512], in_=oo0)
                oo1 = owork.tile([128, 256], BF16, name="oo1", tag="oo1")
                nc.vector.tensor_copy(out=oo1, in_=po_nt0[1])
                nc.sync.dma_start(out=out_d.ap()[0:128, 512:768], in_=oo1)

            # ---- projection nt 1..7 (psS/psE banks now free) ----------
            with tc.tile_pool(name="psO", bufs=2, space="PSUM") as psOp:
                for nt in range(1, NT):
                    pso = psOp.tile([128, C], F32, name="pso", tag="pso")
                    sl = slice(nt * 128, (nt + 1) * 128)
                    for ct in range(CT):
                        lhsT = attnT[:, ct, sl]
                        nc.tensor.matmul(pso[:, 0:512], lhsT,
                                         pwT[:, ct, 0:512],
                                         start=(ct == 0), stop=False)
                        nc.tensor.matmul(pso[:, 512:768], lhsT,
                                         pwT[:, ct, 512:768],
                                         start=(ct == 0), stop=False)
                    nc.tensor.matmul(pso[:, 0:512], ones, pbb[0:1, 0:512],
                                     start=False, stop=True)
                    nc.tensor.matmul(pso[:, 512:768], ones,
                                     pbb[0:1, 512:768],
                                     start=False, stop=True)
                    oo0 = owork.tile([128, 512], BF16, name="oo0", tag="oo0")
                    oo1 = owork.tile([128, 256], BF16, name="oo1", tag="oo1")
                    if nt == NT - 1:
                        nc.vector.tensor_copy(out=oo0, in_=pso[:, 0:512])
                        nc.scalar.copy(out=oo1, in_=pso[:, 512:768])
                    else:
                        nc.scalar.copy(out=oo0, in_=pso[:, 0:512])
                        nc.vector.tensor_copy(out=oo1, in_=pso[:, 512:768])
                    nc.sync.dma_start(out=out_d.ap()[sl, 0:512], in_=oo0)
                    nc.sync.dma_start(out=out_d.ap()[sl, 512:768], in_=oo1)


def _host_prep_shared(qkv_w, qkv_b, proj_w, proj_b):
    wT = np.ascontiguousarray(qkv_w.T).astype(NPBF16)          # [C, 3C]
    pwT = np.ascontiguousarray(proj_w.T).astype(NPBF16)        # [C, C]
    blocks = []
    for ts in (0, 6):
        A = qkv_w[ts * 128:(ts + 1) * 128, :]          # [128c, C]
        blocks.append(np.ascontiguousarray(
            A.T.reshape(CT, 128, 128).transpose(1, 0, 2).reshape(128, C)))
    wqk0 = np.concatenate(blocks, axis=1).astype(NPBF16)   # [128, 2C]
    qb_col = np.ascontiguousarray(
        qkv_b[:2 * C].reshape(12, 128).T).astype(np.float32)
    vb_bcast = np.broadcast_to(qkv_b[2 * C:].astype(NPBF16),
                               (128, C)).copy()
    pb_bcast = np.broadcast_to(proj_b.astype(NPBF16), (128, C)).copy()
    return wT, wqk0, pwT, qb_col, vb_bcast, pb_bcast


def kernel(x, mask, qkv_w, qkv_b, proj_w, proj_b, _trace=False):
    if "nc" not in _CACHE:
        _CACHE["nc"] = _build_bass()
    nc = _CACHE["nc"]

    wT, wqk0, pwT, qb_col, vb_bcast, pb_bcast = _host_prep_shared(
        np.asarray(qkv_w), np.asarray(qkv_b), np.asarray(proj_w),
        np.asarray(proj_b))
    x = np.asarray(x)
    mask = np.asarray(mask)

    in_maps = []
    for i in range(B):
        in_maps.append({
            "xT": np.ascontiguousarray(x[i].T).astype(NPBF16),
            "maskT": np.ascontiguousarray(mask[i].T).astype(NPBF16),
            "qkv_wT": wT,
            "wqk0": wqk0,
            "proj_wT": pwT,
            "qb_col": qb_col,
            "vb_bcast": vb_bcast,
            "pb_bcast": pb_bcast,
        })
    res = run_bass_kernel_spmd(nc, in_maps, core_ids=list(range(B)),
                               trace=_trace)
    out = np.stack([np.asarray(res.results[i]["out"], dtype=np.float32)
                    for i in range(B)], axis=0)
    if _trace:
        _CACHE["last_results"] = res
    return out
